# revision 1
# baseline (speedup 1.0000x reference)
"""Trainium2 Bass kernel for a 2-layer GATv2 (nn_GAT_40372692582770).

Gather-free, PE-centric design (no custom GPSIMD ucode needed):
  - Nodes partitioned by dst range across 8 cores; edges (+self loops)
    routed to the dst owner, sorted by dst, grouped into 128-dst strips,
    padded per strip to a uniform B 128-edge blocks (shared program).
  - Host ships, per layer, per-edge feature columns (the "halo exchange"
    materialized host-side, since the graph is static):
      xsT [65, S]  = x[src_e] columns + ones row   (fp16)
      xdT [65, S]  = x[dst_e] columns + ones row   (fp16)
      xe  [S, 66]  = x[src_e] rows + ones col      (fp16, edge-major)
  - Device per chunk:
      zT[c,e]   = Wl_ext^T xs + Wr_ext^T xd    (PE, feature-major, PSUM)
      L = relu(zT) (ACT), Z = zT (DVE copy); leaky_relu folded as
      e[e,h]    = L^T @ (0.8 A) + Z^T @ (0.2 A)  (PE, per 128-edge block)
      w         = exp(e - 2)                   (ACT, fp16)
      oh[e,s]   = (dstloc == iota)             (DVE), pads dstloc=255 -> 0
      Woh[e,2s] = oh * w_h                     (DVE, both heads stacked)
      GT[j,2s] += xe_block^T @ Woh_block       (PE, per strip, PSUM)
      strip:  out[s, 130] = GT_h0^T @ R2_0 + GT_h1^T @ R2_1  (PE)
              cols = [num_h0 | num_h1 | den_0 | den_1]
  - Finalize: alpha-normalize, head-mean, +bias, gelu -> out_raw fp32 +
    out_act fp16. Host glue (concat/transpose/np-take only) between layers.

One program serves both layers (weights are inputs); compiled once.
"""
import os
import sys
import time

sys.path.insert(0, "/opt/trn_rl_repo")

import numpy as np

import concourse.bass as bass
import concourse.mybir as mybir
import concourse.tile as tile
from concourse import bacc
from concourse.bass_utils import run_bass_kernel_spmd

class Cfg:
    N = 100000
    D = 64
    H = 2
    C = 64
    NCORES = 8
    SPC = 2            # strips per chunk
    ESHIFT = -2.0      # exp bias

    @property
    def RN(self):
        return self.N // self.NCORES

    @property
    def NSTRIP(self):
        return (self.RN + 127) // 128

    @property
    def NSTRIP_PAD(self):
        return ((self.NSTRIP + self.SPC - 1) // self.SPC) * self.SPC

    @property
    def HC(self):
        return self.H * self.C


CFG = Cfg()
FP16 = mybir.dt.float16
FP32 = mybir.dt.float32
AF = mybir.ActivationFunctionType
ALU = mybir.AluOpType


# ------------------------------------------------------------- host prep
def _prep_edges(cfg, src, dst):
    """Route+sort edges, pad strips to B blocks. Returns (B, per-core list of
    (srcids [S], dstloc [S]) in slot order; pad slots src=-1 dstloc=255)."""
    RN = cfg.RN
    core = dst // RN
    per_core = []
    maxblk = 1
    for c in range(cfg.NCORES):
        sel = np.flatnonzero(core == c)
        d = (dst[sel] - c * RN).astype(np.int64)
        s = src[sel].astype(np.int64)
        o = np.argsort(d, kind="stable")
        d, s = d[o], s[o]
        cnt = np.bincount(d >> 7, minlength=cfg.NSTRIP)
        maxblk = max(maxblk, int((cnt.max() + 127) // 128))
        per_core.append((s, d, cnt))
    B = maxblk
    nslot = cfg.NSTRIP_PAD * B * 128
    out = []
    for c in range(cfg.NCORES):
        s, d, cnt = per_core[c]
        sids = np.full(nslot, -1, np.int64)
        dloc = np.full(nslot, 255, np.int64)
        pos = 0
        for st in range(cfg.NSTRIP):
            k = int(cnt[st])
            base = st * B * 128
            sids[base:base + k] = s[pos:pos + k]
            dloc[base:base + k] = d[pos:pos + k] & 127
            pos += k
        out.append((sids, dloc))
    return B, out


def _prep_layer_weights(cfg, Wl, bl, Wr, br, att, bias):
    D, H, C = cfg.D, cfg.H, cfg.C
    HC = cfg.HC
    # Wsl/Wsr: [D+1, 128] projection for zT (feature-major lhsT)
    wsl = np.zeros((D + 1, HC), np.float64)
    wsl[:D] = Wl
    wsl[D] = bl
    wsr = np.zeros((D + 1, HC), np.float64)
    wsr[:D] = Wr
    wsr[D] = br
    # A [128, 2]: att dot, split for lrelu = 0.8*relu(z) + 0.2*z
    A = np.zeros((HC, H), np.float64)
    for h in range(H):
        A[h * C:(h + 1) * C, h] = att[h]
    # R2_h [66, 130]: second-level aggregation weights
    R2 = np.zeros((H, 66, 130), np.float64)
    for h in range(H):
        R2[h, :D, h * C:(h + 1) * C] = Wl[:, h * C:(h + 1) * C]
        R2[h, D, h * C:(h + 1) * C] = bl[h * C:(h + 1) * C]
        R2[h, D, HC + h] = 1.0          # denominator column
    biasF = np.tile(bias.astype(np.float32)[None, :], (128, 1))
    return {
        "wsl": wsl.astype(np.float16), "wsr": wsr.astype(np.float16),
        "A1": (0.8 * A).astype(np.float16), "A2": (0.2 * A).astype(np.float16),
        "R2_0": R2[0].astype(np.float16), "R2_1": R2[1].astype(np.float16),
        "biasF": biasF,
    }


# --------------------------------------------------------- program build
def build_program(cfg, B):
    D, H, C = cfg.D, cfg.H, cfg.C
    HC = cfg.HC
    NBLK = cfg.NSTRIP_PAD * B
    NCHUNK = cfg.NSTRIP_PAD // cfg.SPC
    CB = cfg.SPC * B                   # blocks per chunk
    CS = CB * 128                      # slots per chunk
    S = NBLK * 128
    RROW = HC + 2                      # strip psum row [num128 | d0 d1]
    NS = cfg.NSTRIP_PAD

    nc = bacc.Bacc("TRN2", target_bir_lowering=False, debug=False,
                   num_devices=cfg.NCORES)

    xsT = nc.declare_dram_parameter("xsT", [D + 1, S], FP16, isOutput=False)
    xdT = nc.declare_dram_parameter("xdT", [D + 1, S], FP16, isOutput=False)
    xe = nc.declare_dram_parameter("xe", [S, 66], FP16, isOutput=False)
    wsl = nc.declare_dram_parameter("wsl", [D + 1, HC], FP16, isOutput=False)
    wsr = nc.declare_dram_parameter("wsr", [D + 1, HC], FP16, isOutput=False)
    Amat1 = nc.declare_dram_parameter("A1", [HC, H], FP16, isOutput=False)
    Amat2 = nc.declare_dram_parameter("A2", [HC, H], FP16, isOutput=False)
    R2_0 = nc.declare_dram_parameter("R2_0", [66, 130], FP16, isOutput=False)
    R2_1 = nc.declare_dram_parameter("R2_1", [66, 130], FP16, isOutput=False)
    dstloc = nc.declare_dram_parameter("dstloc", [128, NBLK], FP16, isOutput=False)
    iotaF = nc.declare_dram_parameter("iotaF", [128, 128], FP16, isOutput=False)
    biasF = nc.declare_dram_parameter("biasF", [128, C], FP32, isOutput=False)
    out_raw = nc.declare_dram_parameter("out_raw", [cfg.NSTRIP * 128, C], FP32,
                                        isOutput=True)
    out_act = nc.declare_dram_parameter("out_act", [cfg.NSTRIP * 128, C], FP16,
                                        isOutput=True)

    with tile.TileContext(nc) as tc:
        with (
            tc.tile_pool(name="const", bufs=1) as cpool,
            tc.tile_pool(name="stash", bufs=1) as stpool,
        ):
            wsl_t = cpool.tile([D + 1, HC], FP16)
            nc.sync.dma_start(out=wsl_t[:], in_=wsl[:, :])
            wsr_t = cpool.tile([D + 1, HC], FP16)
            nc.sync.dma_start(out=wsr_t[:], in_=wsr[:, :])
            A1_t = cpool.tile([HC, H], FP16)
            nc.sync.dma_start(out=A1_t[:], in_=Amat1[:, :])
            A2_t = cpool.tile([HC, H], FP16)
            nc.sync.dma_start(out=A2_t[:], in_=Amat2[:, :])
            r2_t = [cpool.tile([66, 130], FP16, tag=f"r2{h}", name=f"r2{h}") for h in range(H)]
            nc.sync.dma_start(out=r2_t[0][:], in_=R2_0[:, :])
            nc.sync.dma_start(out=r2_t[1][:], in_=R2_1[:, :])
            dl_t = cpool.tile([128, NBLK], FP16)
            nc.sync.dma_start(out=dl_t[:], in_=dstloc[:, :])
            iota_t = cpool.tile([128, 128], FP16)
            nc.sync.dma_start(out=iota_t[:], in_=iotaF[:, :])
            ebias_t = cpool.tile([128, 1], FP32)
            nc.vector.memset(ebias_t[:], cfg.ESHIFT)

            stash = stpool.tile([128, NS * (HC + 2)], FP32)
            sv = stash[:].rearrange("p (s w) -> p s w", w=HC + 2)

            with (
                tc.tile_pool(name="eg", bufs=2) as egpool,
                tc.tile_pool(name="ez", bufs=2) as ezpool,
                tc.tile_pool(name="esm", bufs=3) as smpool,
                tc.tile_pool(name="zps", bufs=2, space="PSUM") as zpspool,
                tc.tile_pool(name="eps", bufs=2, space="PSUM") as epspool,
                tc.tile_pool(name="gps", bufs=2, space="PSUM") as gpspool,
                tc.tile_pool(name="sps", bufs=2, space="PSUM") as spspool,
            ):
                for ch in range(NCHUNK):
                    c0 = ch * CS
                    b0 = ch * CB
                    xs_t = egpool.tile([D + 1, CS], FP16, tag="xs")
                    nc.sync.dma_start(out=xs_t[:], in_=xsT[:, c0:c0 + CS])
                    xd_t = egpool.tile([D + 1, CS], FP16, tag="xd")
                    nc.sync.dma_start(out=xd_t[:], in_=xdT[:, c0:c0 + CS])
                    xe_t = egpool.tile([128, CB * 66], FP16, tag="xe")
                    nc.sync.dma_start(
                        out=xe_t[:].rearrange("p (b w) -> p b w", w=66),
                        in_=xe[c0:c0 + CS, :].rearrange("(b p) w -> p b w", p=128))
                    xev = xe_t[:].rearrange("p (b w) -> p b w", w=66)

                    # zT feature-major in groups of <=512 edges
                    L = ezpool.tile([128, CS], FP16, tag="L")
                    Z = ezpool.tile([128, CS], FP16, tag="Z")
                    ngrp = (CS + 511) // 512
                    for g in range(ngrp):
                        g0 = g * 512
                        gw = min(512, CS - g0)
                        zp = zpspool.tile([128, 512], FP32, tag="zp")
                        nc.tensor.matmul(zp[:, :gw], lhsT=wsl_t[:],
                                         rhs=xs_t[:, g0:g0 + gw],
                                         start=True, stop=False)
                        nc.tensor.matmul(zp[:, :gw], lhsT=wsr_t[:],
                                         rhs=xd_t[:, g0:g0 + gw],
                                         start=False, stop=True)
                        nc.scalar.activation(out=L[:, g0:g0 + gw],
                                             in_=zp[:, :gw], func=AF.Relu)
                        nc.vector.tensor_copy(Z[:, g0:g0 + gw], zp[:, :gw])

                    # e-dot per block -> e psum [128, 2*CB]
                    ep = epspool.tile([128, 2 * CB], FP32, tag="ep")
                    for b in range(CB):
                        nc.tensor.matmul(ep[:, 2 * b:2 * b + 2],
                                         lhsT=L[:, b * 128:(b + 1) * 128],
                                         rhs=A1_t[:], start=True, stop=False)
                        nc.tensor.matmul(ep[:, 2 * b:2 * b + 2],
                                         lhsT=Z[:, b * 128:(b + 1) * 128],
                                         rhs=A2_t[:], start=False, stop=True)
                    w_t = smpool.tile([128, 2 * CB], FP16, tag="w")
                    wv = w_t[:].rearrange("p (b k) -> p b k", k=2)
                    nc.scalar.activation(out=w_t[:], in_=ep[:], func=AF.Exp,
                                         bias=ebias_t[:])

                    # onehot + Woh [128, CB, 256]
                    oh = ezpool.tile([128, CB * 128], FP16, tag="oh")
                    ohv = oh[:].rearrange("p (b s) -> p b s", s=128)
                    nc.vector.tensor_tensor(
                        out=ohv[:, :, :],
                        in0=dl_t[:, b0:b0 + CB].unsqueeze(2).to_broadcast([128, CB, 128]),
                        in1=iota_t[:].unsqueeze(1).to_broadcast([128, CB, 128]),
                        op=ALU.is_equal)
                    woh = ezpool.tile([128, CB * 256], FP16, tag="woh")
                    wohv = woh[:].rearrange("p (b s) -> p b s", s=256)
                    for h in range(H):
                        nc.vector.tensor_tensor(
                            out=wohv[:, :, h * 128:(h + 1) * 128],
                            in0=ohv[:, :, :],
                            in1=wv[:, :, h:h + 1].to_broadcast([128, CB, 128]),
                            op=ALU.mult)

                    # GT per strip + strip-level matmuls
                    for s3 in range(cfg.SPC):
                        st = ch * cfg.SPC + s3
                        gt = gpspool.tile([66, 256], FP32, tag="gt")
                        for b in range(B):
                            blk = s3 * B + b
                            nc.tensor.matmul(
                                gt[:], lhsT=xev[:, blk, :],
                                rhs=wohv[:, blk, :],
                                start=(b == 0), stop=(b == B - 1))
                        gts = smpool.tile([66, 256], FP16, tag="gts")
                        eng = nc.vector if (s3 % 2 == 0) else nc.scalar
                        if eng is nc.vector:
                            nc.vector.tensor_copy(gts[:], gt[:])
                        else:
                            nc.scalar.copy(gts[:], gt[:])
                        sp = spspool.tile([128, HC + 2], FP32, tag="sp")
                        nc.tensor.matmul(sp[:], lhsT=gts[:, 0:128], rhs=r2_t[0][:],
                                         start=True, stop=False)
                        nc.tensor.matmul(sp[:], lhsT=gts[:, 128:256], rhs=r2_t[1][:],
                                         start=False, stop=True)
                        eng2 = nc.scalar if (s3 % 2 == 0) else nc.vector
                        dst_sl = stash[:, st * (HC + 2):(st + 1) * (HC + 2)]
                        if eng2 is nc.vector:
                            nc.vector.tensor_copy(dst_sl, sp[:])
                        else:
                            nc.scalar.copy(dst_sl, sp[:])

            # ---------------- finalize ----------------
            with tc.tile_pool(name="fin", bufs=1) as fpool:
                bias_t = fpool.tile([128, C], FP32)
                nc.sync.dma_start(out=bias_t[:], in_=biasF[:, :])
                rec = fpool.tile([128, NS * 2], FP32, tag="rec")
                recv = rec[:].rearrange("p (s k) -> p s k", k=2)
                nc.vector.reciprocal(out=recv[:, :, :], in_=sv[:, :, HC:HC + 2])
                tmean = fpool.tile([128, NS * C], FP32, tag="tmean")
                tm = tmean[:].rearrange("p (s c) -> p s c", c=C)
                nc.vector.tensor_tensor(
                    out=tm[:, :, :], in0=sv[:, :, 0:C],
                    in1=recv[:, :, 0:1].to_broadcast([128, NS, C]), op=ALU.mult)
                t2 = fpool.tile([128, NS * C], FP32, tag="t2")
                t2v = t2[:].rearrange("p (s c) -> p s c", c=C)
                nc.vector.tensor_tensor(
                    out=t2v[:, :, :], in0=sv[:, :, C:2 * C],
                    in1=recv[:, :, 1:2].to_broadcast([128, NS, C]), op=ALU.mult)
                nc.vector.tensor_tensor(out=tm[:, :, :], in0=tm[:, :, :],
                                        in1=t2v[:, :, :], op=ALU.add)
                nc.vector.tensor_scalar_mul(tm[:, :, :], tm[:, :, :], 0.5)
                nc.vector.tensor_tensor(
                    out=tm[:, :, :], in0=tm[:, :, :],
                    in1=bias_t[:].unsqueeze(1).to_broadcast([128, NS, C]),
                    op=ALU.add)
                outg = fpool.tile([128, NS * C], FP16, tag="outg")
                ogv = outg[:].rearrange("p (s c) -> p s c", c=C)
                # gelu_tanh(x) = x * sigmoid(2*sqrt(2/pi)*(x+0.044715 x^3))
                cub = fpool.tile([128, NS * C], FP32, tag="t2")
                cv = cub[:].rearrange("p (s c) -> p s c", c=C)
                nc.scalar.square(cv[:, :, :], tm[:, :, :])
                nc.vector.tensor_tensor(out=cv[:, :, :], in0=cv[:, :, :],
                                        in1=tm[:, :, :], op=ALU.mult)
                nc.vector.tensor_scalar_mul(cv[:, :, :], cv[:, :, :], 0.044715)
                nc.vector.tensor_tensor(out=cv[:, :, :], in0=cv[:, :, :],
                                        in1=tm[:, :, :], op=ALU.add)
                nc.scalar.activation(out=cv[:, :, :], in_=cv[:, :, :],
                                     func=AF.Sigmoid, scale=1.5957691216057308)
                nc.vector.tensor_tensor(out=ogv[:, :, :], in0=cv[:, :, :],
                                        in1=tm[:, :, :], op=ALU.mult)
                NSr = cfg.NSTRIP
                nc.sync.dma_start(
                    out=out_raw[:, :].rearrange("(s p) c -> p s c", p=128),
                    in_=tm[:, :NSr, :])
                nc.sync.dma_start(
                    out=out_act[:, :].rearrange("(s p) c -> p s c", p=128),
                    in_=ogv[:, :NSr, :])

    nc.compile()
    return nc




# ----------------------------------------------------- persistent runner
class Runner:
    """Jit-compiled SPMD callable with reusable device inputs (no donation)."""

    def __init__(self, nc, n_cores):
        import jax
        import concourse.mybir as mb
        from concourse import bass2jax
        from jax.experimental.shard_map import shard_map
        from jax.sharding import Mesh, PartitionSpec
        bass2jax.install_neuronx_cc_hook()
        self.nc = nc
        self.n_cores = n_cores
        in_names, out_names, out_avals, zero_outs = [], [], [], []
        for alloc in nc.m.functions[0].allocations:
            if not isinstance(alloc, mb.MemoryLocationSet):
                continue
            name = alloc.memorylocations[0].name
            if alloc.kind == "ExternalInput":
                in_names.append(name)
            elif alloc.kind == "ExternalOutput":
                out_names.append(name)
                shape = tuple(alloc.tensor_shape)
                dtype = mb.dt.np(alloc.dtype)
                out_avals.append(jax.core.ShapedArray(shape, dtype))
                zero_outs.append(np.zeros(shape, dtype))
        pt = nc.partition_id_tensor
        self.pname = pt.name if pt else None
        if self.pname in in_names:
            in_names.remove(self.pname)
        self.in_names = list(in_names)
        self.out_names = list(out_names)
        self.out_avals = out_avals
        self.zero_outs = zero_outs
        all_in = list(in_names) + list(out_names)
        if self.pname:
            all_in.append(self.pname)

        def _body(*args):
            operands = list(args)
            if self.pname:
                operands.append(bass2jax.partition_id_tensor())
            outs = bass2jax._bass_exec_p.bind(
                *operands,
                out_avals=tuple(out_avals),
                in_names=tuple(all_in),
                out_names=tuple(out_names),
                lowering_input_output_aliases=(),
                sim_require_finite=True,
                sim_require_nnan=True,
                nc=nc,
            )
            return tuple(outs)

        devices = jax.devices()[:n_cores]
        self.mesh = Mesh(np.asarray(devices), ("core",))
        np_in = (PartitionSpec("core"),) * (len(in_names) + len(out_names))
        np_out = (PartitionSpec("core"),) * len(out_names)
        self.fn = jax.jit(shard_map(_body, mesh=self.mesh, in_specs=np_in,
                                    out_specs=np_out, check_rep=False),
                          keep_unused=True)

    def put(self, in_maps):
        """Concat per-core inputs and move to device. Returns arg list."""
        import jax
        from jax.sharding import NamedSharding, PartitionSpec
        sh = NamedSharding(self.mesh, PartitionSpec("core"))
        args = []
        for name in self.in_names:
            cat = np.concatenate([np.asarray(m[name]) for m in in_maps], axis=0)
            args.append(jax.device_put(cat, sh))
        for z in self.zero_outs:
            zz = np.zeros((self.n_cores * z.shape[0], *z.shape[1:]), z.dtype)
            args.append(jax.device_put(zz, sh))
        return args

    def run(self, args):
        return self.fn(*args)

    def results(self, out_arrs):
        res = []
        for c in range(self.n_cores):
            res.append({
                name: np.asarray(out_arrs[i]).reshape(
                    self.n_cores, *self.out_avals[i].shape)[c]
                for i, name in enumerate(self.out_names)})
        return res


# ------------------------------------------------------------- kernel()
_CACHE = {}
_RUNNERS = {}
LAST_ARGS = None
LAST_LAUNCH_NS = None


def build_in_map(cfg, cur, slots, dstid, dlocP, lw, iotaF, c):
    S = len(slots[c][0])
    sids, _ = slots[c]
    pad = sids < 0
    xs = cur[np.where(pad, 0, sids)]
    xs[pad] = 0
    xsT = np.empty((cfg.D + 1, S), np.float16)
    xsT[:cfg.D] = xs.T
    xsT[cfg.D] = (~pad).astype(np.float16)
    xe_arr = np.zeros((S, 66), np.float16)
    xe_arr[:, :cfg.D] = xs
    xe_arr[:, cfg.D] = (~pad).astype(np.float16)
    dg = dstid[c]
    padd = dg < 0
    xd = cur[c * cfg.RN + np.where(padd, 0, np.minimum(dg, cfg.RN - 1))]
    xd[padd] = 0
    xdT = np.empty((cfg.D + 1, S), np.float16)
    xdT[:cfg.D] = xd.T
    xdT[cfg.D] = (~padd).astype(np.float16)
    return {
        "xsT": xsT, "xdT": xdT, "xe": xe_arr,
        "wsl": lw["wsl"], "wsr": lw["wsr"], "A1": lw["A1"], "A2": lw["A2"],
        "R2_0": lw["R2_0"], "R2_1": lw["R2_1"],
        "dstloc": dlocP[c], "iotaF": iotaF, "biasF": lw["biasF"],
    }


def prep_all(cfg, src, dst):
    B, slots = _prep_edges(cfg, src, dst)
    S = cfg.NSTRIP_PAD * B * 128
    dstid = []
    dlocP = []
    for c in range(cfg.NCORES):
        sids, dloc = slots[c]
        strip = (np.arange(S) // (B * 128))
        dstid.append(np.where(sids < 0, -1, strip * 128 + dloc))
        dlocP.append(np.ascontiguousarray(
            dloc.reshape(cfg.NSTRIP_PAD * B, 128).T.astype(np.float16)))
    return B, slots, dstid, dlocP


def kernel(embeded_nodes_features, edges_connectivity, Wl, bl, Wr, br, att, bias):
    global LAST_LAUNCH_NS
    cfg = CFG
    x = np.asarray(embeded_nodes_features, np.float32)
    ec = np.asarray(edges_connectivity)
    src = np.concatenate([ec[0], np.arange(cfg.N, dtype=ec.dtype)]).astype(np.int64)
    dst = np.concatenate([ec[1], np.arange(cfg.N, dtype=ec.dtype)]).astype(np.int64)
    Wl = np.asarray(Wl, np.float32)
    bl = np.asarray(bl, np.float32)
    Wr = np.asarray(Wr, np.float32)
    br = np.asarray(br, np.float32)
    att = np.asarray(att, np.float32)
    bias = np.asarray(bias, np.float32)
    L = Wl.shape[0]

    B, slots, dstid, dlocP = prep_all(cfg, src, dst)
    if B not in _CACHE:
        _CACHE[B] = build_program(cfg, B)
    prog = _CACHE[B]
    lws = [_prep_layer_weights(cfg, Wl[i], bl[i], Wr[i], br[i], att[i], bias[i])
           for i in range(L)]
    iotaF = np.tile(np.arange(128, dtype=np.float16)[None, :], (128, 1))

    cur = x.astype(np.float16)
    out_full = None
    _launch_ns = []
    _args_hist = []
    for i in range(L):
        lw = lws[i]
        in_maps = [build_in_map(cfg, cur, slots, dstid, dlocP, lw, iotaF, c)
                   for c in range(cfg.NCORES)]
        if B not in _RUNNERS:
            _RUNNERS[B] = Runner(prog, cfg.NCORES)
        runner = _RUNNERS[B]
        args = runner.put(in_maps)
        _args_hist.append(args)
        t0 = time.time()
        outs = runner.run(args)
        import jax
        jax.block_until_ready(outs)
        _launch_ns.append(int((time.time() - t0) * 1e9))
        res = runner.results(outs)
        raw = np.concatenate(
            [res[c]["out_raw"][:cfg.RN] for c in range(cfg.NCORES)], axis=0)
        actv = np.concatenate(
            [res[c]["out_act"][:cfg.RN] for c in range(cfg.NCORES)], axis=0)
        out_full = raw
        cur = actv
    LAST_LAUNCH_NS = _launch_ns
    global LAST_ARGS
    LAST_ARGS = _args_hist
    return out_full.astype(np.float32)



# revision 10
# speedup vs baseline: 1.0958x; 1.0958x over previous
"""Trainium2 Bass kernel for a 2-layer GATv2 (nn_GAT_40372692582770).

Gather-free, PE-centric design (no custom GPSIMD ucode needed):
  - Nodes partitioned by dst range across 8 cores; edges (+self loops)
    routed to the dst owner, sorted by dst, grouped into 128-dst strips,
    padded per strip to a uniform B 128-edge blocks (shared program).
  - Host ships, per layer, per-edge feature columns (the "halo exchange"
    materialized host-side, since the graph is static):
      xsd [128, S] = [x[src_e].T ; x[dst_e].T]  (fp16, feature-major)
      xe  [128, NBLK*66] = x[src_e] rows + ones col, pre-tiled per
                           128-edge block (fp16, edge-major)
  - Device per chunk:
      zT[c,e]  = Wsd^T xsd                        (PE, one matmul, K=128)
      L2       = prelu(zT + zbias, 0.2)           (ACT, fp16)
      e[e,h]   = L2_blk^T @ A                     (PE, per 128-edge block)
      w        = exp(e - 2)                       (ACT, fp16)
      oh       = (dl == iota)                     (DVE) -- 32-wide windows
                 for blocks b>=1 (sorted edges span <= 32 dsts/block),
                 full 128-wide for b==0 (doubles as PSUM zero-init)
      Woh_h    = oh * w_h                         (DVE)
      GT[j,·] += xe_blk^T @ Woh_blk               (PE, windowed accumulate
                 into per-strip PSUM at baked col offsets)
      strip:  out[s, 130] = GT_h0^T @ R2_0 + GT_h1^T @ R2_1  (PE)
              cols = [num_h0 | num_h1 | den_0 | den_1]
  - Finalize: alpha-normalize, head-mean, +bias, gelu -> out_raw fp32 +
    out_act fp16. Host glue (concat/transpose/np-take only) between layers.

One program serves both layers (weights are inputs); compiled once per
(B, window-offset table). Window offsets are shared across cores (SPMD).
"""
import os
import sys
import time

sys.path.insert(0, "/opt/trn_rl_repo")

import numpy as np

import concourse.bass as bass
import concourse.mybir as mybir
import concourse.tile as tile
from concourse import bacc
from concourse.bass_utils import run_bass_kernel_spmd

class Cfg:
    N = 100000
    D = 64
    H = 2
    C = 64
    NCORES = 8
    SPC = 2            # strips per chunk
    W = 40             # one-hot window width for blocks b>=1
    ESHIFT = -2.0      # exp bias

    @property
    def RN(self):
        return self.N // self.NCORES

    @property
    def NSTRIP(self):
        return (self.RN + 127) // 128

    @property
    def NSTRIP_PAD(self):
        return ((self.NSTRIP + self.SPC - 1) // self.SPC) * self.SPC

    @property
    def HC(self):
        return self.H * self.C


CFG = Cfg()
FP16 = mybir.dt.float16
FP32 = mybir.dt.float32
AF = mybir.ActivationFunctionType
ALU = mybir.AluOpType


# ------------------------------------------------------------- host prep
def _prep_edges(cfg, src, dst):
    """Route+sort edges, pad strips to B blocks. Returns (B, per-core list of
    (srcids [S], dstloc [S]) in slot order; pad slots src=-1 dstloc=255)."""
    RN = cfg.RN
    core = dst // RN
    per_core = []
    maxblk = 1
    for c in range(cfg.NCORES):
        sel = np.flatnonzero(core == c)
        d = (dst[sel] - c * RN).astype(np.int64)
        s = src[sel].astype(np.int64)
        o = np.argsort(d, kind="stable")
        d, s = d[o], s[o]
        cnt = np.bincount(d >> 7, minlength=cfg.NSTRIP)
        maxblk = max(maxblk, int((cnt.max() + 127) // 128))
        per_core.append((s, d, cnt))
    B = maxblk
    nslot = cfg.NSTRIP_PAD * B * 128
    out = []
    for c in range(cfg.NCORES):
        s, d, cnt = per_core[c]
        sids = np.full(nslot, -1, np.int64)
        dloc = np.full(nslot, 255, np.int64)
        pos = 0
        for st in range(cfg.NSTRIP):
            k = int(cnt[st])
            base = st * B * 128
            sids[base:base + k] = s[pos:pos + k]
            dloc[base:base + k] = d[pos:pos + k] & 127
            pos += k
        out.append((sids, dloc))
    return B, out


def _window_offsets(cfg, B, slots):
    """Per (strip, block>=1) window start, shared across cores: the min dl
    over all cores' real edges in that block, clamped to [0, 128-W]."""
    NS = cfg.NSTRIP_PAD
    lo = np.full((NS, B), 999, np.int64)
    hi = np.full((NS, B), -1, np.int64)
    for c in range(cfg.NCORES):
        _, dloc = slots[c]
        dv = dloc.reshape(NS, B, 128)
        real = dv != 255
        dm = np.where(real, dv, 999).min(axis=2)
        dM = np.where(real, dv, -1).max(axis=2)
        lo = np.minimum(lo, dm)
        hi = np.maximum(hi, dM)
    off = np.clip(lo, 0, 128 - cfg.W)
    off[lo == 999] = 0
    off[:, 0] = 0  # block 0 uses the full 128-wide one-hot
    if os.environ.get("GAT_NOWIN"):
        off[:, :] = 0
    span = (hi - off)[:, 1:]
    if (span >= cfg.W).any():
        raise RuntimeError(f"window W={cfg.W} too narrow: span {span.max()+1}")
    return off.astype(np.int64)


def _prep_layer_weights(cfg, Wl, bl, Wr, br, att, bias):
    D, H, C = cfg.D, cfg.H, cfg.C
    HC = cfg.HC
    wsd = np.zeros((2 * D, HC), np.float64)
    wsd[:D] = Wl
    wsd[D:] = Wr
    A = np.zeros((HC, H), np.float64)
    for h in range(H):
        A[h * C:(h + 1) * C, h] = att[h]
    # R2_h [66, 130]: second-level aggregation weights
    R2 = np.zeros((H, 66, 130), np.float64)
    for h in range(H):
        R2[h, :D, h * C:(h + 1) * C] = Wl[:, h * C:(h + 1) * C]
        R2[h, D, h * C:(h + 1) * C] = bl[h * C:(h + 1) * C]
        R2[h, D, HC + h] = 1.0          # denominator column
    biasF = np.tile(bias.astype(np.float32)[None, :], (128, 1))
    zbias = (bl + br).astype(np.float32).reshape(HC, 1)
    return {
        "wsd": wsd.astype(np.float16),
        "A": A.astype(np.float16),
        "R2_0": R2[0].astype(np.float16), "R2_1": R2[1].astype(np.float16),
        "biasF": biasF, "zbias": zbias,
    }


# --------------------------------------------------------- program build
def build_program(cfg, B, off_tab):
    D, H, C = cfg.D, cfg.H, cfg.C
    HC = cfg.HC
    W = cfg.W
    NBLK = cfg.NSTRIP_PAD * B
    NCHUNK = cfg.NSTRIP_PAD // cfg.SPC
    CB = cfg.SPC * B                   # blocks per chunk
    CS = CB * 128                      # slots per chunk
    S = NBLK * 128
    NS = cfg.NSTRIP_PAD

    nc = bacc.Bacc("TRN2", target_bir_lowering=False, debug=False,
                   num_devices=cfg.NCORES)

    xsd = nc.declare_dram_parameter("xsd", [2 * D, S], FP16, isOutput=False)
    xe = nc.declare_dram_parameter("xe", [128, NBLK * 66], FP16, isOutput=False)
    wsd = nc.declare_dram_parameter("wsd", [2 * D, HC], FP16, isOutput=False)
    Amat = nc.declare_dram_parameter("A", [HC, H], FP16, isOutput=False)
    R2_0 = nc.declare_dram_parameter("R2_0", [66, 130], FP16, isOutput=False)
    R2_1 = nc.declare_dram_parameter("R2_1", [66, 130], FP16, isOutput=False)
    dstloc = nc.declare_dram_parameter("dstloc", [128, NBLK], FP16, isOutput=False)
    iotaF = nc.declare_dram_parameter("iotaF", [128, 128], FP16, isOutput=False)
    biasF = nc.declare_dram_parameter("biasF", [128, C], FP32, isOutput=False)
    zbias = nc.declare_dram_parameter("zbias", [HC, 1], FP32, isOutput=False)
    out_raw = nc.declare_dram_parameter("out_raw", [cfg.NSTRIP * 128, C], FP32,
                                        isOutput=True)
    out_act = nc.declare_dram_parameter("out_act", [cfg.NSTRIP * 128, C], FP16,
                                        isOutput=True)
    DEBUG = bool(os.environ.get("GAT_DEBUG"))
    if DEBUG:
        dbg_L2 = nc.declare_dram_parameter("dbg_L2", [128, S], FP16, isOutput=True)
        dbg_w = nc.declare_dram_parameter("dbg_w", [128, 2 * NBLK], FP16,
                                          isOutput=True)
        dbg_gts = nc.declare_dram_parameter("dbg_gts", [66, NS * 256], FP16,
                                            isOutput=True)

    with tile.TileContext(nc) as tc:
        with (
            tc.tile_pool(name="const", bufs=1) as cpool,
            tc.tile_pool(name="stash", bufs=1) as stpool,
        ):
            wsd_t = cpool.tile([2 * D, HC], FP16)
            nc.sync.dma_start(out=wsd_t[:], in_=wsd[:, :])
            A_t = cpool.tile([HC, H], FP16)
            nc.sync.dma_start(out=A_t[:], in_=Amat[:, :])
            r2_t = [cpool.tile([66, 130], FP16, tag=f"r2{h}", name=f"r2{h}") for h in range(H)]
            nc.sync.dma_start(out=r2_t[0][:], in_=R2_0[:, :])
            nc.sync.dma_start(out=r2_t[1][:], in_=R2_1[:, :])
            dl_t = cpool.tile([128, NBLK], FP16)
            nc.sync.dma_start(out=dl_t[:], in_=dstloc[:, :])
            iota_t = cpool.tile([128, 128], FP16)
            nc.sync.dma_start(out=iota_t[:], in_=iotaF[:, :])
            zbias_t = cpool.tile([HC, 1], FP32)
            nc.sync.dma_start(out=zbias_t[:], in_=zbias[:, :])
            ebias_t = cpool.tile([128, 1], FP32)
            nc.vector.memset(ebias_t[:], cfg.ESHIFT)

            stash = stpool.tile([128, NS * (HC + 2)], FP32)
            sv = stash[:].rearrange("p (s w) -> p s w", w=HC + 2)

            with (
                tc.tile_pool(name="eg", bufs=2) as egpool,
                tc.tile_pool(name="ez", bufs=2) as ezpool,
                tc.tile_pool(name="esm", bufs=3) as smpool,
                tc.tile_pool(name="zps", bufs=2, space="PSUM") as zpspool,
                tc.tile_pool(name="eps", bufs=2, space="PSUM") as epspool,
                tc.tile_pool(name="gps", bufs=2, space="PSUM") as gpspool,
                tc.tile_pool(name="sps", bufs=2, space="PSUM") as spspool,
            ):
                for ch in range(NCHUNK):
                    c0 = ch * CS
                    b0 = ch * CB
                    xsd_t = egpool.tile([2 * D, CS], FP16, tag="xsd")
                    nc.sync.dma_start(out=xsd_t[:], in_=xsd[:, c0:c0 + CS])
                    xe_t = egpool.tile([128, CB * 66], FP16, tag="xe")
                    nc.sync.dma_start(out=xe_t[:], in_=xe[:, b0 * 66:(b0 + CB) * 66])
                    xev = xe_t[:].rearrange("p (b w) -> p b w", w=66)

                    # zT feature-major in groups of <=512 edges; L2 = prelu
                    L2 = ezpool.tile([128, CS], FP16, tag="L2")
                    ngrp = (CS + 511) // 512
                    for g in range(ngrp):
                        g0 = g * 512
                        gw = min(512, CS - g0)
                        zp = zpspool.tile([128, 512], FP32, tag="zp")
                        nc.tensor.matmul(zp[:, :gw], lhsT=wsd_t[:],
                                         rhs=xsd_t[:, g0:g0 + gw],
                                         start=True, stop=True)
                        nc.scalar.activation(out=L2[:, g0:g0 + gw],
                                             in_=zp[:, :gw], func=AF.Prelu,
                                             alpha=0.2, bias=zbias_t[:])

                    # e-dot per block -> e psum [128, 2*CB]
                    ep = epspool.tile([128, 2 * CB], FP32, tag="ep")
                    for b in range(CB):
                        nc.tensor.matmul(ep[:, 2 * b:2 * b + 2],
                                         lhsT=L2[:, b * 128:(b + 1) * 128],
                                         rhs=A_t[:], start=True, stop=True)
                    w_t = smpool.tile([128, 2 * CB], FP16, tag="w")
                    wv = w_t[:].rearrange("p (b k) -> p b k", k=2)
                    nc.scalar.activation(out=w_t[:], in_=ep[:], func=AF.Exp,
                                         bias=ebias_t[:])
                    if DEBUG:
                        nc.sync.dma_start(out=dbg_L2[:, c0:c0 + CS], in_=L2[:])
                        nc.sync.dma_start(out=dbg_w[:, 2 * b0:2 * (b0 + CB)],
                                          in_=w_t[:])

                    # one-hot: windowed (W wide) for all blocks + full for b0s
                    ohW = ezpool.tile([128, CB * W], FP16, tag="ohW")
                    ohWv = ohW[:].rearrange("p (b s) -> p b s", s=W)
                    nc.vector.tensor_tensor(
                        out=ohWv[:, :, :],
                        in0=dl_t[:, b0:b0 + CB].unsqueeze(2).to_broadcast([128, CB, W]),
                        in1=iota_t[:, 0:W].unsqueeze(1).to_broadcast([128, CB, W]),
                        op=ALU.is_equal)
                    ohF = ezpool.tile([128, cfg.SPC * 128], FP16, tag="ohF")
                    for s3 in range(cfg.SPC):
                        gb = b0 + s3 * B
                        nc.vector.tensor_tensor(
                            out=ohF[:, s3 * 128:(s3 + 1) * 128].unsqueeze(1),
                            in0=dl_t[:, gb:gb + 1].unsqueeze(2).to_broadcast([128, 1, 128]),
                            in1=iota_t[:].unsqueeze(1).to_broadcast([128, 1, 128]),
                            op=ALU.is_equal)

                    # Woh = oh * w_h (windowed for b>=1, full for b0)
                    wohW = [ezpool.tile([128, CB * W], FP16, tag=f"wohW{h}",
                                        name=f"wohW{h}") for h in range(H)]
                    wohF = [ezpool.tile([128, cfg.SPC * 128], FP16, tag=f"wohF{h}",
                                        name=f"wohF{h}") for h in range(H)]
                    for h in range(H):
                        wWv = wohW[h][:].rearrange("p (b s) -> p b s", s=W)
                        nc.vector.tensor_tensor(
                            out=wWv[:, :, :], in0=ohWv[:, :, :],
                            in1=wv[:, :, h:h + 1].to_broadcast([128, CB, W]),
                            op=ALU.mult)
                        for s3 in range(cfg.SPC):
                            nc.vector.tensor_tensor(
                                out=wohF[h][:, s3 * 128:(s3 + 1) * 128].unsqueeze(1),
                                in0=ohF[:, s3 * 128:(s3 + 1) * 128].unsqueeze(1),
                                in1=wv[:, s3 * B:s3 * B + 1, h:h + 1]
                                    .to_broadcast([128, 1, 128]),
                                op=ALU.mult)

                    # GT per strip (windowed accumulate) + strip-level matmuls
                    for s3 in range(cfg.SPC):
                        st = ch * cfg.SPC + s3
                        gt = gpspool.tile([66, 256], FP32, tag="gt")
                        for b in range(B):
                            blk = s3 * B + b
                            last = (b == B - 1)
                            if b == 0:
                                # start=True clears has_written for the WHOLE
                                # bank -- issue it exactly once per gt tile;
                                # later matmuls overwrite-on-first-touch /
                                # accumulate-after via per-element bits.
                                for h in range(H):
                                    nc.tensor.matmul(
                                        gt[:, h * 128:(h + 1) * 128],
                                        lhsT=xev[:, blk, :],
                                        rhs=wohF[h][:, s3 * 128:(s3 + 1) * 128],
                                        start=(h == 0), stop=False,
                                        skip_group_check=True)
                            else:
                                off = int(off_tab[st, b])
                                for h in range(H):
                                    nc.tensor.matmul(
                                        gt[:, h * 128 + off:h * 128 + off + W],
                                        lhsT=xev[:, blk, :],
                                        rhs=wohW[h][:, blk * W:(blk + 1) * W],
                                        start=False, stop=last and h == H - 1,
                                        skip_group_check=True)
                        gts = smpool.tile([66, 256], FP16, tag="gts")
                        eng = nc.vector if (s3 % 2 == 0) else nc.scalar
                        if eng is nc.vector:
                            nc.vector.tensor_copy(gts[:], gt[:])
                        else:
                            nc.scalar.copy(gts[:], gt[:])
                        if DEBUG:
                            nc.sync.dma_start(
                                out=dbg_gts[:, st * 256:(st + 1) * 256],
                                in_=gts[:])
                        sp = spspool.tile([128, HC + 2], FP32, tag="sp")
                        nc.tensor.matmul(sp[:], lhsT=gts[:, 0:128], rhs=r2_t[0][:],
                                         start=True, stop=False)
                        nc.tensor.matmul(sp[:], lhsT=gts[:, 128:256], rhs=r2_t[1][:],
                                         start=False, stop=True)
                        eng2 = nc.scalar if (s3 % 2 == 0) else nc.vector
                        dst_sl = stash[:, st * (HC + 2):(st + 1) * (HC + 2)]
                        if eng2 is nc.vector:
                            nc.vector.tensor_copy(dst_sl, sp[:])
                        else:
                            nc.scalar.copy(dst_sl, sp[:])

            # ---------------- finalize ----------------
            with tc.tile_pool(name="fin", bufs=1) as fpool:
                bias_t = fpool.tile([128, C], FP32)
                nc.sync.dma_start(out=bias_t[:], in_=biasF[:, :])
                rec = fpool.tile([128, NS * 2], FP32, tag="rec")
                recv = rec[:].rearrange("p (s k) -> p s k", k=2)
                nc.vector.reciprocal(out=recv[:, :, :], in_=sv[:, :, HC:HC + 2])
                tmean = fpool.tile([128, NS * C], FP32, tag="tmean")
                tm = tmean[:].rearrange("p (s c) -> p s c", c=C)
                nc.vector.tensor_tensor(
                    out=tm[:, :, :], in0=sv[:, :, 0:C],
                    in1=recv[:, :, 0:1].to_broadcast([128, NS, C]), op=ALU.mult)
                t2 = fpool.tile([128, NS * C], FP32, tag="t2")
                t2v = t2[:].rearrange("p (s c) -> p s c", c=C)
                nc.vector.tensor_tensor(
                    out=t2v[:, :, :], in0=sv[:, :, C:2 * C],
                    in1=recv[:, :, 1:2].to_broadcast([128, NS, C]), op=ALU.mult)
                nc.vector.tensor_tensor(out=tm[:, :, :], in0=tm[:, :, :],
                                        in1=t2v[:, :, :], op=ALU.add)
                nc.vector.tensor_scalar_mul(tm[:, :, :], tm[:, :, :], 0.5)
                nc.vector.tensor_tensor(
                    out=tm[:, :, :], in0=tm[:, :, :],
                    in1=bias_t[:].unsqueeze(1).to_broadcast([128, NS, C]),
                    op=ALU.add)
                outg = fpool.tile([128, NS * C], FP16, tag="outg")
                ogv = outg[:].rearrange("p (s c) -> p s c", c=C)
                # gelu_tanh(x) = x * sigmoid(2*sqrt(2/pi)*(x+0.044715 x^3))
                cub = fpool.tile([128, NS * C], FP32, tag="t2")
                cv = cub[:].rearrange("p (s c) -> p s c", c=C)
                nc.scalar.square(cv[:, :, :], tm[:, :, :])
                nc.vector.tensor_tensor(out=cv[:, :, :], in0=cv[:, :, :],
                                        in1=tm[:, :, :], op=ALU.mult)
                nc.vector.tensor_scalar_mul(cv[:, :, :], cv[:, :, :], 0.044715)
                nc.vector.tensor_tensor(out=cv[:, :, :], in0=cv[:, :, :],
                                        in1=tm[:, :, :], op=ALU.add)
                nc.scalar.activation(out=cv[:, :, :], in_=cv[:, :, :],
                                     func=AF.Sigmoid, scale=1.5957691216057308)
                nc.vector.tensor_tensor(out=ogv[:, :, :], in0=cv[:, :, :],
                                        in1=tm[:, :, :], op=ALU.mult)
                NSr = cfg.NSTRIP
                nc.sync.dma_start(
                    out=out_raw[:, :].rearrange("(s p) c -> p s c", p=128),
                    in_=tm[:, :NSr, :])
                nc.sync.dma_start(
                    out=out_act[:, :].rearrange("(s p) c -> p s c", p=128),
                    in_=ogv[:, :NSr, :])

    nc.compile()
    return nc




# ----------------------------------------------------- persistent runner
class Runner:
    """Jit-compiled SPMD callable with reusable device inputs (no donation)."""

    def __init__(self, nc, n_cores):
        import jax
        import concourse.mybir as mb
        from concourse import bass2jax
        from jax.experimental.shard_map import shard_map
        from jax.sharding import Mesh, PartitionSpec
        bass2jax.install_neuronx_cc_hook()
        self.nc = nc
        self.n_cores = n_cores
        in_names, out_names, out_avals, zero_outs = [], [], [], []
        for alloc in nc.m.functions[0].allocations:
            if not isinstance(alloc, mb.MemoryLocationSet):
                continue
            name = alloc.memorylocations[0].name
            if alloc.kind == "ExternalInput":
                in_names.append(name)
            elif alloc.kind == "ExternalOutput":
                out_names.append(name)
                shape = tuple(alloc.tensor_shape)
                dtype = mb.dt.np(alloc.dtype)
                out_avals.append(jax.core.ShapedArray(shape, dtype))
                zero_outs.append(np.zeros(shape, dtype))
        pt = nc.partition_id_tensor
        self.pname = pt.name if pt else None
        if self.pname in in_names:
            in_names.remove(self.pname)
        self.in_names = list(in_names)
        self.out_names = list(out_names)
        self.out_avals = out_avals
        self.zero_outs = zero_outs
        all_in = list(in_names) + list(out_names)
        if self.pname:
            all_in.append(self.pname)

        def _body(*args):
            operands = list(args)
            if self.pname:
                operands.append(bass2jax.partition_id_tensor())
            outs = bass2jax._bass_exec_p.bind(
                *operands,
                out_avals=tuple(out_avals),
                in_names=tuple(all_in),
                out_names=tuple(out_names),
                lowering_input_output_aliases=(),
                sim_require_finite=True,
                sim_require_nnan=True,
                nc=nc,
            )
            return tuple(outs)

        devices = jax.devices()[:n_cores]
        self.mesh = Mesh(np.asarray(devices), ("core",))
        np_in = (PartitionSpec("core"),) * (len(in_names) + len(out_names))
        np_out = (PartitionSpec("core"),) * len(out_names)
        self.fn = jax.jit(shard_map(_body, mesh=self.mesh, in_specs=np_in,
                                    out_specs=np_out, check_rep=False),
                          keep_unused=True)

    def put(self, in_maps):
        """Concat per-core inputs and move to device. Returns arg list."""
        import jax
        from jax.sharding import NamedSharding, PartitionSpec
        sh = NamedSharding(self.mesh, PartitionSpec("core"))
        args = []
        for name in self.in_names:
            cat = np.concatenate([np.asarray(m[name]) for m in in_maps], axis=0)
            args.append(jax.device_put(cat, sh))
        for z in self.zero_outs:
            zz = np.zeros((self.n_cores * z.shape[0], *z.shape[1:]), z.dtype)
            args.append(jax.device_put(zz, sh))
        return args

    def run(self, args):
        return self.fn(*args)

    def results(self, out_arrs):
        res = []
        for c in range(self.n_cores):
            res.append({
                name: np.asarray(out_arrs[i]).reshape(
                    self.n_cores, *self.out_avals[i].shape)[c]
                for i, name in enumerate(self.out_names)})
        return res


# ------------------------------------------------------------- kernel()
_CACHE = {}
_RUNNERS = {}
LAST_ARGS = None
LAST_LAUNCH_NS = None


def build_in_map(cfg, cur, slots, dstid, dlP, lw, iotaF, c):
    S = len(slots[c][0])
    sids, _ = slots[c]
    pad = sids < 0
    xs = cur[np.where(pad, 0, sids)]
    xs[pad] = 0
    dg = dstid[c]
    padd = dg < 0
    xd = cur[c * cfg.RN + np.where(padd, 0, np.minimum(dg, cfg.RN - 1))]
    xd[padd] = 0
    xsd = np.empty((2 * cfg.D, S), np.float16)
    xsd[:cfg.D] = xs.T
    xsd[cfg.D:] = xd.T
    NBLK = S // 128
    xe_arr = np.zeros((S, 66), np.float16)
    xe_arr[:, :cfg.D] = xs
    xe_arr[:, cfg.D] = (~pad).astype(np.float16)
    xe_tl = np.ascontiguousarray(
        xe_arr.reshape(NBLK, 128, 66).transpose(1, 0, 2).reshape(128, NBLK * 66))
    return {
        "xsd": xsd, "xe": xe_tl,
        "wsd": lw["wsd"], "A": lw["A"],
        "R2_0": lw["R2_0"], "R2_1": lw["R2_1"],
        "dstloc": dlP[c], "iotaF": iotaF, "biasF": lw["biasF"],
        "zbias": lw["zbias"],
    }


def prep_all(cfg, src, dst):
    B, slots = _prep_edges(cfg, src, dst)
    off_tab = _window_offsets(cfg, B, slots)
    S = cfg.NSTRIP_PAD * B * 128
    NBLK = S // 128
    strip = (np.arange(S) // (B * 128))
    blk_in_strip = (np.arange(S) // 128) % B
    off_slot = off_tab[strip, blk_in_strip]
    dstid = []
    dlP = []
    for c in range(cfg.NCORES):
        sids, dloc = slots[c]
        dstid.append(np.where(sids < 0, -1, strip * 128 + dloc))
        rel = np.where(dloc == 255, 255, dloc - off_slot)
        assert rel.min() >= 0 and ((rel == 255) | (rel < 128)).all()
        dlP.append(np.ascontiguousarray(
            rel.reshape(NBLK, 128).T.astype(np.float16)))
    return B, off_tab, slots, dstid, dlP


def kernel(embeded_nodes_features, edges_connectivity, Wl, bl, Wr, br, att, bias):
    global LAST_LAUNCH_NS
    cfg = CFG
    x = np.asarray(embeded_nodes_features, np.float32)
    ec = np.asarray(edges_connectivity)
    src = np.concatenate([ec[0], np.arange(cfg.N, dtype=ec.dtype)]).astype(np.int64)
    dst = np.concatenate([ec[1], np.arange(cfg.N, dtype=ec.dtype)]).astype(np.int64)
    Wl = np.asarray(Wl, np.float32)
    bl = np.asarray(bl, np.float32)
    Wr = np.asarray(Wr, np.float32)
    br = np.asarray(br, np.float32)
    att = np.asarray(att, np.float32)
    bias = np.asarray(bias, np.float32)
    L = Wl.shape[0]

    if os.environ.get("GAT_NOWIN"):
        cfg.W = 128
    B, off_tab, slots, dstid, dlP = prep_all(cfg, src, dst)
    key = (B, off_tab.tobytes())
    if key not in _CACHE:
        _CACHE[key] = build_program(cfg, B, off_tab)
    prog = _CACHE[key]
    lws = [_prep_layer_weights(cfg, Wl[i], bl[i], Wr[i], br[i], att[i], bias[i])
           for i in range(L)]
    iotaF = np.tile(np.arange(128, dtype=np.float16)[None, :], (128, 1))

    cur = x.astype(np.float16)
    out_full = None
    _launch_ns = []
    _args_hist = []
    for i in range(L):
        lw = lws[i]
        in_maps = [build_in_map(cfg, cur, slots, dstid, dlP, lw, iotaF, c)
                   for c in range(cfg.NCORES)]
        if key not in _RUNNERS:
            _RUNNERS[key] = Runner(prog, cfg.NCORES)
        runner = _RUNNERS[key]
        args = runner.put(in_maps)
        _args_hist.append(args)
        t0 = time.time()
        outs = runner.run(args)
        import jax
        jax.block_until_ready(outs)
        _launch_ns.append(int((time.time() - t0) * 1e9))
        res = runner.results(outs)
        raw = np.concatenate(
            [res[c]["out_raw"][:cfg.RN] for c in range(cfg.NCORES)], axis=0)
        actv = np.concatenate(
            [res[c]["out_act"][:cfg.RN] for c in range(cfg.NCORES)], axis=0)
        out_full = raw
        cur = actv
    LAST_LAUNCH_NS = _launch_ns
    global LAST_ARGS
    LAST_ARGS = _args_hist
    return out_full.astype(np.float32)
